# revision 5
# baseline (speedup 1.0000x reference)
"""Trainium2 Bass kernel for nn_ComplexMixture.

Reference:
  output_real[b,n,m] = sum_s w[b,s] * (r[b,s,n]*r[b,s,m] + i[b,s,n]*i[b,s,m])
  output_imag[b,n,m] = sum_s w[b,s] * (i[b,s,n]*r[b,s,m] - r[b,s,n]*i[b,s,m])

Shapes: B=32, S=128, N=256, fp32. w is uniform [0,1) so sqrt(w) is real.

out_r is symmetric and out_i is antisymmetric, so the device only computes
  P = out_r + out_i
and the host recovers out_r = (P + P^T)/2, out_i = (P - P^T)/2.
The host pre-scales the inputs: Yr = sqrt(w)[:,None]*r, Yi = sqrt(w)[:,None]*i
(pure input preprocessing, O(B*S*N)) and casts them to bf16. With
U = Yr - Yi, V = Yr + Yi:
  P[n,m] = sum_s Yr[s,n]*U[s,m] + Yi[s,n]*V[s,m]
i.e. per 128-row output chunk c:  P_c = Yr_c.T @ U + Yi_c.T @ V  (PSUM accum).

bf16: matmul streams 1 cycle/row (vs 4 for fp32), input DMA bytes halve,
PSUM accumulates fp32, PSUM->SBUF copy casts to bf16 so output bytes halve.
Max rel err ~4e-3, within the 2e-2 gate.

Schedule notes (from NTFF traces):
 - DMA trigger->first-byte latency is ~1.5-1.9us, so all input DMAs fire in
   the first instructions of the body, per-batch so each batch's compute
   unblocks as early as possible (sync: b0,b1 / scalar: b2,b3 / none on
   SWDGE: gpsimd triggers cost ~650ns and SWDGE queues drain late).
 - No Activation-engine compute ops: an ACT_TABLE_LOAD (~1.3us) would block
   the scalar HWDGE queue right when the input DMAs should trigger.
 - UV and the PSUM->bf16 cast alternate between gpsimd and vector so no
   single engine exceeds the PE's ~0.65us/batch pace.
 - Warmup matmuls ramp the PE clock during the input DMA dead time only
   (~6 of them); more just delays the real matmuls.
"""

import os

import numpy as np
import ml_dtypes

import concourse.bass as bass
import concourse.mybir as mybir
import concourse.tile as tile
from concourse import bacc
from concourse.bass_utils import run_bass_kernel_spmd

B, S, N = 32, 128, 256
NCORES = 8
BPC = B // NCORES  # batches per core
XCOL = 2 * N * BPC

F32 = mybir.dt.float32
BF16 = mybir.dt.bfloat16
N_WARMUP = int(os.environ.get("CM_WARMUP", "6"))
# 1: skip the PSUM->SBUF cast; DMA fp32 straight from PSUM to DRAM.
PSUM_DMA = os.environ.get("CM_PSUM_DMA", "0") == "1"

LAST_RESULTS = None  # stashed BassKernelResults for test harness introspection


def build_nc() -> bass.Bass:
    nc = bacc.Bacc(num_swdge_queues=1)
    xin = nc.dram_tensor("xpack", [S, XCOL], BF16, kind="ExternalInput")
    out_dt = F32 if PSUM_DMA else BF16
    out = nc.dram_tensor("out_all", [BPC, 128, 2, N], out_dt, kind="ExternalOutput")

    with tile.TileContext(nc) as tc:
        with (
            tc.tile_pool(name="io", bufs=1) as io_pool,
            tc.tile_pool(name="yp", bufs=BPC) as y_pool,
            tc.tile_pool(name="op", bufs=BPC) as out_pool,
            tc.tile_pool(name="ps", bufs=BPC, space="PSUM") as ps_pool,
            tc.tile_pool(name="wu", bufs=1, space="PSUM") as wu_pool,
        ):
            X_all = io_pool.tile([S, XCOL], BF16, tag="X", name="X_all")

            # Input DMAs first: per-batch, 2 per HWDGE ring, in batch order.
            for b in range(BPC):
                bsl = slice(b * 2 * N, (b + 1) * 2 * N)
                eng = nc.sync if b < 2 else nc.scalar
                eng.dma_start(out=X_all[:, bsl], in_=xin[:, bsl])

            # PE warmup during the input-DMA dead time.
            if N_WARMUP:
                junk = io_pool.tile([S, N], BF16, tag="junk", name="junk")
                nc.gpsimd.memset(junk, 1.0)
                wups = wu_pool.tile([128, N], F32, tag="wu", name="wups")
                for k in range(N_WARMUP):
                    nc.tensor.matmul(
                        wups, lhsT=junk[:, 0:128], rhs=junk,
                        start=True, stop=True, skip_group_check=True,
                    )

            for b in range(BPC):
                X = X_all[:, b * 2 * N : (b + 1) * 2 * N]
                Yr = X[:, 0:N]
                Yi = X[:, N : 2 * N]
                UV = y_pool.tile([S, 2 * N], BF16, tag="UV", name=f"UV{b}")
                nc.gpsimd.tensor_sub(UV[:, 0:N], Yr, Yi)
                nc.gpsimd.tensor_add(UV[:, N : 2 * N], Yr, Yi)

                ps = ps_pool.tile([128, 2 * N], F32, tag="ps", name=f"ps{b}")
                for c in range(2):
                    csl = slice(c * 128, c * 128 + 128)
                    osl = slice(c * N, (c + 1) * N)
                    nc.tensor.matmul(ps[:, osl], lhsT=Yr[:, csl], rhs=UV[:, 0:N], start=True, stop=False)
                    nc.tensor.matmul(ps[:, osl], lhsT=Yi[:, csl], rhs=UV[:, N : 2 * N], start=False, stop=True)

                dst = out[b].rearrange("p c m -> p (c m)")
                if PSUM_DMA:
                    # fp32 straight out of PSUM on alternating HWDGE rings;
                    # tail batch split across both rings for a parallel drain.
                    if b == BPC - 1:
                        nc.sync.dma_start(out=out[b][:, 0, :], in_=ps[:, 0:N])
                        nc.scalar.dma_start(out=out[b][:, 1, :], in_=ps[:, N : 2 * N])
                    else:
                        eng = nc.sync if b % 2 == 0 else nc.scalar
                        eng.dma_start(out=dst, in_=ps)
                    continue

                O = out_pool.tile([128, 2 * N], BF16, tag="O", name=f"O{b}")
                if b == BPC - 1:
                    # Tail batch: split cast + DMA into halves on both rings
                    # so the final drain is parallel.
                    nc.vector.tensor_copy(O[:, 0:N], ps[:, 0:N])
                    nc.sync.dma_start(out=out[b][:, 0, :], in_=O[:, 0:N])
                    nc.vector.tensor_copy(O[:, N : 2 * N], ps[:, N : 2 * N])
                    nc.scalar.dma_start(out=out[b][:, 1, :], in_=O[:, N : 2 * N])
                else:
                    nc.vector.tensor_copy(O, ps)
                    eng = nc.sync if b % 2 == 0 else nc.scalar
                    eng.dma_start(out=dst, in_=O)
    nc.compile()
    return nc


def kernel(**inputs: np.ndarray):
    global LAST_RESULTS
    r = np.asarray(inputs["input_real"], dtype=np.float32)
    i = np.asarray(inputs["input_imag"], dtype=np.float32)
    w = np.ascontiguousarray(np.asarray(inputs["weight"], dtype=np.float32))
    assert r.shape == (B, S, N) and i.shape == (B, S, N) and w.shape == (B, S)

    # [B, 2, S, N] -> per-core [S, (b t n)] batch-major blocks, bf16
    sws = np.sqrt(w)  # [B, S]
    xin = np.stack([r, i], axis=1) * sws[:, None, :, None]  # pre-scaled
    xin = xin.astype(ml_dtypes.bfloat16)

    in_maps = []
    for c in range(NCORES):
        sl = slice(c * BPC, (c + 1) * BPC)
        xpack = np.transpose(xin[sl], (2, 0, 1, 3)).reshape(S, 2 * N * BPC)
        in_maps.append({"xpack": np.ascontiguousarray(xpack)})

    nc = build_nc()
    res = run_bass_kernel_spmd(nc, in_maps, core_ids=list(range(NCORES)))
    LAST_RESULTS = res

    out_all = np.concatenate(
        [np.asarray(res.results[c]["out_all"]).astype(np.float32) for c in range(NCORES)],
        axis=0,
    )  # [B, 128, 2, N]; P[b, c*128+p, m] = out_all[b, p, c, m]
    P = np.transpose(out_all, (0, 2, 1, 3)).reshape(B, N, N)
    Pt = np.transpose(P, (0, 2, 1))
    out_r = (P + Pt) * np.float32(0.5)
    out_i = (P - Pt) * np.float32(0.5)
    return (np.ascontiguousarray(out_r), np.ascontiguousarray(out_i))


# revision 9
# speedup vs baseline: 1.0494x; 1.0494x over previous
"""Trainium2 Bass kernel for nn_ComplexMixture.

Reference:
  output_real[b,n,m] = sum_s w[b,s] * (r[b,s,n]*r[b,s,m] + i[b,s,n]*i[b,s,m])
  output_imag[b,n,m] = sum_s w[b,s] * (i[b,s,n]*r[b,s,m] - r[b,s,n]*i[b,s,m])

Shapes: B=32, S=128, N=256, fp32. w is uniform [0,1) so sqrt(w) is real.

out_r is symmetric and out_i is antisymmetric, so the device only computes
  P = out_r + out_i
and the host recovers out_r = (P + P^T)/2, out_i = (P - P^T)/2.
The host pre-scales the inputs: Yr = sqrt(w)[:,None]*r, Yi = sqrt(w)[:,None]*i
(pure input preprocessing, O(B*S*N)) and casts them to bf16. With
U = Yr - Yi, V = Yr + Yi:
  P[n,m] = sum_s Yr[s,n]*U[s,m] + Yi[s,n]*V[s,m]
i.e. per 128-row output chunk c:  P_c = Yr_c.T @ U + Yi_c.T @ V  (PSUM accum).

bf16: matmul streams 1 cycle/row (vs 4 for fp32), input DMA bytes halve,
PSUM accumulates fp32, PSUM->SBUF copy casts to bf16 so output bytes halve.
Max rel err ~4e-3, within the 2e-2 gate.

Schedule notes (from NTFF traces):
 - DMA trigger->first-byte latency is ~1.5-1.9us, so all input DMAs fire in
   the first instructions of the body, per-batch so each batch's compute
   unblocks as early as possible (sync: b0,b1 / scalar: b2,b3 / none on
   SWDGE: gpsimd triggers cost ~650ns and SWDGE queues drain late).
 - No Activation-engine compute ops: an ACT_TABLE_LOAD (~1.3us) would block
   the scalar HWDGE queue right when the input DMAs should trigger.
 - UV and the PSUM->bf16 cast alternate between gpsimd and vector so no
   single engine exceeds the PE's ~0.65us/batch pace.
 - Warmup matmuls ramp the PE clock during the input DMA dead time only
   (~6 of them); more just delays the real matmuls.
"""

import os

import numpy as np
import ml_dtypes

import concourse.bass as bass
import concourse.mybir as mybir
import concourse.tile as tile
from concourse import bacc
from concourse.bass_utils import run_bass_kernel_spmd

B, S, N = 32, 128, 256
NCORES = 8
BPC = B // NCORES  # batches per core
XCOL = 2 * N * BPC

F32 = mybir.dt.float32
BF16 = mybir.dt.bfloat16
N_WARMUP = int(os.environ.get("CM_WARMUP", "14"))

LAST_RESULTS = None  # stashed BassKernelResults for test harness introspection


def build_nc() -> bass.Bass:
    nc = bacc.Bacc(num_swdge_queues=1)
    xin = nc.dram_tensor("xpack", [S, XCOL], BF16, kind="ExternalInput")
    out = nc.dram_tensor("out_all", [BPC, 128, 2, N], BF16, kind="ExternalOutput")

    with tile.TileContext(nc) as tc:
        with (
            tc.tile_pool(name="io", bufs=1) as io_pool,
            tc.tile_pool(name="yp", bufs=BPC) as y_pool,
            tc.tile_pool(name="op", bufs=BPC) as out_pool,
            tc.tile_pool(name="ps", bufs=BPC, space="PSUM") as ps_pool,
            tc.tile_pool(name="wu", bufs=1, space="PSUM") as wu_pool,
        ):
            X_all = io_pool.tile([S, XCOL], BF16, tag="X", name="X_all")

            # Input DMAs first, per-batch, in compute order. Sync (SP HWDGE)
            # carries b0..b2; gpsimd (SWDGE) b3. Scalar gets none: the
            # compiler pins an ACT_TABLE_LOAD (~1.3us) at the top of the
            # scalar stream for the cast-copies, which would delay any input
            # DMA behind it.
            for b in range(BPC):
                bsl = slice(b * 2 * N, (b + 1) * 2 * N)
                eng = nc.sync if b < 3 else nc.gpsimd
                eng.dma_start(out=X_all[:, bsl], in_=xin[:, bsl])

            # PE warmup during the input-DMA dead time.
            if N_WARMUP:
                junk = io_pool.tile([S, N], BF16, tag="junk", name="junk")
                nc.gpsimd.memset(junk, 1.0)
                wups = wu_pool.tile([128, N], F32, tag="wu", name="wups")
                for k in range(N_WARMUP):
                    nc.tensor.matmul(
                        wups, lhsT=junk[:, 0:128], rhs=junk,
                        start=True, stop=True, skip_group_check=True,
                    )

            for b in range(BPC):
                X = X_all[:, b * 2 * N : (b + 1) * 2 * N]
                Yr = X[:, 0:N]
                Yi = X[:, N : 2 * N]
                UV = y_pool.tile([S, 2 * N], BF16, tag="UV", name=f"UV{b}")
                nc.vector.tensor_sub(UV[:, 0:N], Yr, Yi)
                nc.vector.tensor_add(UV[:, N : 2 * N], Yr, Yi)

                ps = ps_pool.tile([128, 2 * N], F32, tag="ps", name=f"ps{b}")
                for c in range(2):
                    csl = slice(c * 128, c * 128 + 128)
                    osl = slice(c * N, (c + 1) * N)
                    nc.tensor.matmul(ps[:, osl], lhsT=Yr[:, csl], rhs=UV[:, 0:N], start=True, stop=False)
                    nc.tensor.matmul(ps[:, osl], lhsT=Yi[:, csl], rhs=UV[:, N : 2 * N], start=False, stop=True)

                O = out_pool.tile([128, 2 * N], BF16, tag="O", name=f"O{b}")
                if b == BPC - 1:
                    # Tail batch: split cast + DMA into halves on both rings
                    # so the final drain is parallel.
                    nc.scalar.copy(out=O[:, 0:N], in_=ps[:, 0:N])
                    nc.scalar.dma_start(out=out[b][:, 0, :], in_=O[:, 0:N])
                    nc.vector.tensor_copy(O[:, N : 2 * N], ps[:, N : 2 * N])
                    nc.sync.dma_start(out=out[b][:, 1, :], in_=O[:, N : 2 * N])
                else:
                    if b % 2 == 0:
                        nc.vector.tensor_copy(O, ps)
                    else:
                        nc.scalar.copy(out=O, in_=ps)
                    dst = out[b].rearrange("p c m -> p (c m)")
                    eng = nc.sync if b % 2 == 0 else nc.scalar
                    eng.dma_start(out=dst, in_=O)
    nc.compile()
    return nc


def kernel(**inputs: np.ndarray):
    global LAST_RESULTS
    r = np.asarray(inputs["input_real"], dtype=np.float32)
    i = np.asarray(inputs["input_imag"], dtype=np.float32)
    w = np.ascontiguousarray(np.asarray(inputs["weight"], dtype=np.float32))
    assert r.shape == (B, S, N) and i.shape == (B, S, N) and w.shape == (B, S)

    # [B, 2, S, N] -> per-core [S, (b t n)] batch-major blocks, bf16
    sws = np.sqrt(w)  # [B, S]
    xin = np.stack([r, i], axis=1) * sws[:, None, :, None]  # pre-scaled
    xin = xin.astype(ml_dtypes.bfloat16)

    in_maps = []
    for c in range(NCORES):
        sl = slice(c * BPC, (c + 1) * BPC)
        xpack = np.transpose(xin[sl], (2, 0, 1, 3)).reshape(S, 2 * N * BPC)
        in_maps.append({"xpack": np.ascontiguousarray(xpack)})

    nc = build_nc()
    res = run_bass_kernel_spmd(nc, in_maps, core_ids=list(range(NCORES)))
    LAST_RESULTS = res

    out_all = np.concatenate(
        [np.asarray(res.results[c]["out_all"]).astype(np.float32) for c in range(NCORES)],
        axis=0,
    )  # [B, 128, 2, N]; P[b, c*128+p, m] = out_all[b, p, c, m]
    P = np.transpose(out_all, (0, 2, 1, 3)).reshape(B, N, N)
    Pt = np.transpose(P, (0, 2, 1))
    out_r = (P + Pt) * np.float32(0.5)
    out_i = (P - Pt) * np.float32(0.5)
    return (np.ascontiguousarray(out_r), np.ascontiguousarray(out_i))
